# revision 2
# baseline (speedup 1.0000x reference)
"""Trainium2 Bass kernel for DigitConvolutionalModel.

Math: the 3x3 valid conv on the 28x28 image is a linear map, so it folds into
the first Linear layer:
    out = relu(x @ W_eff + b1) @ w2.T + b2
where W_eff[784, 128] = C @ w1.T and C[784, 676] is the conv-as-matrix built
from conv_w.  W_eff is built on the host (O(1) w.r.t. batch); the device does
the two batch matmuls.

Distribution: pure data parallel — batch dim of x sharded across 8 NeuronCores,
weights replicated.  Each core receives its x shard transposed ([784, 8192],
feature-major) so the contraction dim lands on SBUF partitions with contiguous
DMA, computes out.T [10, 8192], and the host reassembles [65536, 10].
"""

import numpy as np

import concourse.bass as bass  # noqa: F401  (bass registers mybir lowerings)
import concourse.mybir as mybir
import concourse.tile as tile
from concourse import bacc
from concourse.bass_utils import run_bass_kernel_spmd

N_CORES = 8
B = 65536
B_SH = B // N_CORES  # 8192 rows per core
D = 784              # 28*28 input features
H = 128              # hidden
OUT = 10
KT = 112             # contraction tile (SBUF partitions used)
NK = D // KT         # 7 K-tiles
NB = 512             # batch columns per tile (= one fp32 PSUM bank)
NT = B_SH // NB      # 16 batch tiles
# float32r: fp32 with mantissa rounded to 11 bits (low 12 bits zero) — the PE
# runs it at full 1 cycle/row rate at free-dim >= 256 (plain fp32 is 4
# cycles/row).  x and W_eff are pre-rounded on the host, so the DRAM/SBUF
# tensors carry the float32r dtype end-to-end.  mm2 stays plain fp32 (its 16
# small matmuls overlap under the DMA-bound pipeline).

_CACHE = {}


def _build_nc():
    f32 = mybir.dt.float32
    nc = bacc.Bacc("TRN2", target_bir_lowering=False, debug=False,
                   num_devices=N_CORES)
    f32r = mybir.dt.float32r
    xt = nc.dram_tensor("xt", [D, B_SH], f32r, kind="ExternalInput").ap()
    weff = nc.dram_tensor("weff", [D, H], f32r, kind="ExternalInput").ap()
    w2t = nc.dram_tensor("w2t", [H, OUT], f32, kind="ExternalInput").ap()
    b1c = nc.dram_tensor("b1c", [H, 1], f32, kind="ExternalInput").ap()
    b2c = nc.dram_tensor("b2c", [OUT, 1], f32, kind="ExternalInput").ap()
    out = nc.dram_tensor("out", [OUT, B_SH], f32, kind="ExternalOutput").ap()

    relu = mybir.ActivationFunctionType.Relu
    ident = mybir.ActivationFunctionType.Identity

    with tile.TileContext(nc) as tc:
        with (
            tc.tile_pool(name="wpool", bufs=1) as wpool,
            tc.tile_pool(name="xpool", bufs=4) as xpool,
            tc.tile_pool(name="hpool", bufs=3) as hpool,
            tc.tile_pool(name="opool", bufs=3) as opool,
            tc.tile_pool(name="ps1", bufs=4, space="PSUM") as ps1pool,
            tc.tile_pool(name="ps2", bufs=2, space="PSUM") as ps2pool,
        ):
            # Replicated params, loaded once.
            w_sb = wpool.tile([KT, NK, H], f32r)
            nc.sync.dma_start(w_sb[:], weff.rearrange("(k p) m -> p k m", p=KT))
            w2_sb = wpool.tile([H, OUT], f32)
            nc.sync.dma_start(w2_sb[:], w2t[:])
            b1_sb = wpool.tile([H, 1], f32)
            nc.sync.dma_start(b1_sb[:], b1c[:])
            b2_sb = wpool.tile([OUT, 1], f32)
            nc.sync.dma_start(b2_sb[:], b2c[:])

            xt_r = xt.rearrange("(k p) b -> p k b", p=KT)
            for t in range(NT):
                x_sb = xpool.tile([KT, NK, NB], f32r)
                nc.sync.dma_start(x_sb[:], xt_r[:, :, t * NB:(t + 1) * NB])

                # h.T[128, NB] = W_eff.T @ x.T, accumulated over K-tiles.
                ps1 = ps1pool.tile([H, NB], f32)
                for k in range(NK):
                    nc.tensor.matmul(
                        ps1[:],
                        lhsT=w_sb[:, k, :],
                        rhs=x_sb[:, k, :],
                        start=(k == 0),
                        stop=(k == NK - 1),
                    )
                h_sb = hpool.tile([H, NB], f32)
                nc.scalar.activation(h_sb[:], ps1[:], relu, bias=b1_sb[:])

                # out.T[10, NB] = w2 @ h.T
                ps2 = ps2pool.tile([OUT, NB], f32)
                nc.tensor.matmul(
                    ps2[:],
                    lhsT=w2_sb[:],
                    rhs=h_sb[:],
                    start=True,
                    stop=True,
                )
                o_sb = opool.tile([OUT, NB], f32)
                nc.scalar.activation(o_sb[:], ps2[:], ident, bias=b2_sb[:])
                nc.sync.dma_start(out[:, t * NB:(t + 1) * NB], o_sb[:])

    nc.compile()
    return nc


def _get_nc():
    if "nc" not in _CACHE:
        _CACHE["nc"] = _build_nc()
    return _CACHE["nc"]


def _round_fp32r(a: np.ndarray) -> np.ndarray:
    """Round fp32 to the fp32r format: 11 mantissa bits, RNE, low 12 bits zero."""
    u = a.astype(np.float32).view(np.uint32)
    lsb = (u >> np.uint32(12)) & np.uint32(1)
    u = (u + np.uint32(0x7FF) + lsb) & np.uint32(0xFFFFF000)
    return u.view(np.float32)


def _fold_weights(conv_w: np.ndarray, w1: np.ndarray) -> np.ndarray:
    """W_eff[784, 128]: h_pre = x @ W_eff  ==  conv(x) @ w1.T  (float64 accum)."""
    w1k = w1.reshape(H, 26, 26).transpose(1, 2, 0).astype(np.float64)  # [i,j,k]
    cw = conv_w.astype(np.float64)
    W = np.zeros((28, 28, H), np.float64)
    for di in range(3):
        for dj in range(3):
            W[di:di + 26, dj:dj + 26, :] += cw[di, dj] * w1k
    return np.ascontiguousarray(W.reshape(D, H).astype(np.float32))


def make_in_maps(x, conv_w, w1, b1, w2, b2):
    x = np.asarray(x, np.float32)
    weff = _round_fp32r(
        _fold_weights(np.asarray(conv_w, np.float32), np.asarray(w1, np.float32)))
    w2t = np.ascontiguousarray(np.asarray(w2, np.float32).T)          # [128, 10]
    b1c = np.ascontiguousarray(np.asarray(b1, np.float32).reshape(H, 1))
    b2c = np.ascontiguousarray(np.asarray(b2, np.float32).reshape(OUT, 1))
    in_maps = []
    for i in range(N_CORES):
        xt = _round_fp32r(np.ascontiguousarray(x[i * B_SH:(i + 1) * B_SH].T))
        in_maps.append({"xt": xt, "weff": weff, "w2t": w2t, "b1c": b1c, "b2c": b2c})
    return in_maps


def kernel(x, conv_w, w1, b1, w2, b2):
    nc = _get_nc()
    in_maps = make_in_maps(x, conv_w, w1, b1, w2, b2)
    res = run_bass_kernel_spmd(nc, in_maps, list(range(N_CORES)))
    out = np.concatenate([res.results[i]["out"] for i in range(N_CORES)], axis=1)
    return np.ascontiguousarray(out.T)  # [65536, 10] float32


# revision 3
# speedup vs baseline: 1.0175x; 1.0175x over previous
"""Trainium2 Bass kernel for DigitConvolutionalModel.

Math: the 3x3 valid conv on the 28x28 image is a linear map, so it folds into
the first Linear layer:
    out = relu(x @ W_eff + b1) @ w2.T + b2
where W_eff[784, 128] = C @ w1.T and C[784, 676] is the conv-as-matrix built
from conv_w.  W_eff is built on the host (O(1) w.r.t. batch); the device does
the two batch matmuls.

Distribution: pure data parallel — batch dim of x sharded across 8 NeuronCores,
weights replicated.  Each core receives its x shard permuted to a
partition-major layout [112, 16, 7, 512] (partition p, batch-tile t, k-tile k,
column c; feature f = k*112 + p) so every DMA lands on SBUF partitions with
long contiguous DRAM runs.  Each core computes out.T [10, 8192] and the host
reassembles [65536, 10].

dtypes: x and W_eff are pre-rounded on the host to float32r (fp32 with 11
mantissa bits, low 12 bits zero) — the PE runs fp32r matmuls at full rate
(1 cycle/row at free-dim >= 256) vs 4 cycles/row for plain fp32.  The hidden
activation is also produced as float32r by the ScalarE relu so the second
matmul runs at full rate too.
"""

import numpy as np

import concourse.bass as bass  # noqa: F401  (bass registers mybir lowerings)
import concourse.mybir as mybir
import concourse.tile as tile
from concourse import bacc
from concourse.bass_utils import run_bass_kernel_spmd

N_CORES = 8
B = 65536
B_SH = B // N_CORES  # 8192 rows per core
D = 784              # 28*28 input features
H = 128              # hidden
OUT = 10
KT = 112             # contraction tile (SBUF partitions used)
NK = D // KT         # 7 K-tiles
NB = 512             # batch columns per tile (= one fp32 PSUM bank)
NT = B_SH // NB      # 16 batch tiles
G = 2                # batch tiles per x DMA (28KB contiguous per partition)

_CACHE = {}


def _build_nc():
    f32 = mybir.dt.float32
    f32r = mybir.dt.float32r
    nc = bacc.Bacc("TRN2", target_bir_lowering=False, debug=False,
                   num_devices=N_CORES)
    # x, partition-major: [p, t, k, c] with feature f = k*112 + p
    xtp = nc.dram_tensor("xtp", [KT, NT, NK, NB], f32r,
                         kind="ExternalInput").ap()
    weff = nc.dram_tensor("weff", [D, H], f32r, kind="ExternalInput").ap()
    w2t = nc.dram_tensor("w2t", [H, OUT], f32r, kind="ExternalInput").ap()
    b1c = nc.dram_tensor("b1c", [H, 1], f32, kind="ExternalInput").ap()
    b2c = nc.dram_tensor("b2c", [OUT, 1], f32, kind="ExternalInput").ap()
    out = nc.dram_tensor("out", [OUT, B_SH], f32, kind="ExternalOutput").ap()

    relu = mybir.ActivationFunctionType.Relu
    ident = mybir.ActivationFunctionType.Identity

    with tile.TileContext(nc) as tc:
        with (
            tc.tile_pool(name="wpool", bufs=1) as wpool,
            tc.tile_pool(name="xpool", bufs=4) as xpool,
            tc.tile_pool(name="hpool", bufs=4) as hpool,
            tc.tile_pool(name="opool", bufs=4) as opool,
            tc.tile_pool(name="ps1", bufs=5, space="PSUM") as ps1pool,
            tc.tile_pool(name="ps2", bufs=2, space="PSUM") as ps2pool,
        ):
            # Replicated params, loaded once (small; keep off the two HWDGE
            # rings that stream x).
            w_sb = wpool.tile([KT, NK, H], f32r)
            nc.gpsimd.dma_start(w_sb[:], weff.rearrange("(k p) m -> p k m", p=KT))
            w2_sb = wpool.tile([H, OUT], f32r)
            nc.gpsimd.dma_start(w2_sb[:], w2t[:])
            b1_sb = wpool.tile([H, 1], f32)
            nc.gpsimd.dma_start(b1_sb[:], b1c[:])
            b2_sb = wpool.tile([OUT, 1], f32)
            nc.gpsimd.dma_start(b2_sb[:], b2c[:])

            for g in range(NT // G):
                x_sb = xpool.tile([KT, G, NK, NB], f32r)
                # alternate the two HWDGE rings (sync / scalar)
                dma_eng = nc.sync if g % 2 == 0 else nc.scalar
                dma_eng.dma_start(x_sb[:], xtp[:, g * G:(g + 1) * G, :, :])

                for s in range(G):
                    t = g * G + s
                    # h.T[128, NB] = W_eff.T @ x.T, accumulated over K-tiles.
                    ps1 = ps1pool.tile([H, NB], f32)
                    for k in range(NK):
                        nc.tensor.matmul(
                            ps1[:],
                            lhsT=w_sb[:, k, :],
                            rhs=x_sb[:, s, k, :],
                            start=(k == 0),
                            stop=(k == NK - 1),
                        )
                    h_sb = hpool.tile([H, NB], f32r)
                    nc.scalar.activation(h_sb[:], ps1[:], relu, bias=b1_sb[:])

                    # out.T[10, NB] = w2 @ h.T
                    ps2 = ps2pool.tile([OUT, NB], f32)
                    nc.tensor.matmul(ps2[:], lhsT=w2_sb[:], rhs=h_sb[:],
                                     start=True, stop=True)
                    o_sb = opool.tile([OUT, NB], f32)
                    nc.scalar.activation(o_sb[:], ps2[:], ident, bias=b2_sb[:])
                    nc.gpsimd.dma_start(out[:, t * NB:(t + 1) * NB], o_sb[:])

    nc.compile()
    return nc


def _get_nc():
    if "nc" not in _CACHE:
        _CACHE["nc"] = _build_nc()
    return _CACHE["nc"]


def _round_fp32r(a: np.ndarray) -> np.ndarray:
    """Round fp32 to the fp32r format: 11 mantissa bits, RNE, low 12 bits zero."""
    u = np.ascontiguousarray(a, np.float32).view(np.uint32)
    lsb = (u >> np.uint32(12)) & np.uint32(1)
    u = (u + np.uint32(0x7FF) + lsb) & np.uint32(0xFFFFF000)
    return u.view(np.float32)


def _fold_weights(conv_w: np.ndarray, w1: np.ndarray) -> np.ndarray:
    """W_eff[784, 128]: h_pre = x @ W_eff  ==  conv(x) @ w1.T  (float64 accum)."""
    w1k = w1.reshape(H, 26, 26).transpose(1, 2, 0).astype(np.float64)  # [i,j,k]
    cw = conv_w.astype(np.float64)
    W = np.zeros((28, 28, H), np.float64)
    for di in range(3):
        for dj in range(3):
            W[di:di + 26, dj:dj + 26, :] += cw[di, dj] * w1k
    return W.reshape(D, H).astype(np.float32)


def make_in_maps(x, conv_w, w1, b1, w2, b2):
    x = np.asarray(x, np.float32)
    weff = _round_fp32r(
        _fold_weights(np.asarray(conv_w, np.float32), np.asarray(w1, np.float32)))
    w2t = _round_fp32r(np.asarray(w2, np.float32).T)                  # [128, 10]
    b1c = np.ascontiguousarray(np.asarray(b1, np.float32).reshape(H, 1))
    b2c = np.ascontiguousarray(np.asarray(b2, np.float32).reshape(OUT, 1))
    in_maps = []
    for i in range(N_CORES):
        xs = x[i * B_SH:(i + 1) * B_SH]                    # [8192, 784]
        # [t*NB+c, k*KT+p] -> [p, t, k, c]
        xtp = xs.reshape(NT, NB, NK, KT).transpose(3, 0, 2, 1)
        in_maps.append({"xtp": _round_fp32r(xtp), "weff": weff, "w2t": w2t,
                        "b1c": b1c, "b2c": b2c})
    return in_maps


def kernel(x, conv_w, w1, b1, w2, b2):
    nc = _get_nc()
    in_maps = make_in_maps(x, conv_w, w1, b1, w2, b2)
    res = run_bass_kernel_spmd(nc, in_maps, list(range(N_CORES)))
    out = np.concatenate([res.results[i]["out"] for i in range(N_CORES)], axis=1)
    return np.ascontiguousarray(out.T)  # [65536, 10] float32


# revision 4
# speedup vs baseline: 1.6743x; 1.6455x over previous
"""Trainium2 Bass kernel for DigitConvolutionalModel.

Math: the 3x3 valid conv on the 28x28 image is a linear map, so it folds into
the first Linear layer:
    out = relu(x @ W_eff + b1) @ w2.T + b2
where W_eff[784, 128] = C @ w1.T and C[784, 676] is the conv-as-matrix built
from conv_w.  W_eff is built on the host (O(1) w.r.t. batch); the device does
the two batch matmuls.

Distribution: pure data parallel — batch dim of x sharded across 8 NeuronCores,
weights replicated.  Each core receives its x shard permuted to a
partition-major layout [112, 16, 7, 512] (partition p, batch-tile t, k-tile k,
column c; feature f = k*112 + p) so every DMA lands on SBUF partitions with
long contiguous DRAM runs.  Each core computes out.T [10, 8192] and the host
reassembles [65536, 10].

dtypes: x and W_eff ship as fp16 (10 mantissa bits, comparable to the PE's
fp32r path at 11 bits) — halves the HBM traffic, which is the roofline here,
and fp16 matmuls run at the full 1 cycle/row PE rate.  The hidden activation
is produced as float32r (fp32, 11-bit mantissa) by the ScalarE relu so the
second matmul also runs at full rate; accumulation is always fp32 in PSUM.
"""

import numpy as np

import concourse.bass as bass  # noqa: F401  (bass registers mybir lowerings)
import concourse.mybir as mybir
import concourse.tile as tile
from concourse import bacc
from concourse.bass_utils import run_bass_kernel_spmd

N_CORES = 8
B = 65536
B_SH = B // N_CORES  # 8192 rows per core
D = 784              # 28*28 input features
H = 128              # hidden
OUT = 10
KT = 112             # contraction tile (SBUF partitions used)
NK = D // KT         # 7 K-tiles
NB = 512             # batch columns per tile (= one fp32 PSUM bank)
NT = B_SH // NB      # 16 batch tiles
G = 2                # batch tiles per x DMA (28KB contiguous per partition)

_CACHE = {}


def _build_nc():
    f32 = mybir.dt.float32
    f32r = mybir.dt.float32r
    f16 = mybir.dt.float16
    nc = bacc.Bacc("TRN2", target_bir_lowering=False, debug=False,
                   num_devices=N_CORES)
    # x, partition-major: [p, t, k, c] with feature f = k*112 + p
    xtp = nc.dram_tensor("xtp", [KT, NT, NK, NB], f16,
                         kind="ExternalInput").ap()
    weff = nc.dram_tensor("weff", [D, H], f16, kind="ExternalInput").ap()
    w2t = nc.dram_tensor("w2t", [H, OUT], f32r, kind="ExternalInput").ap()
    b1c = nc.dram_tensor("b1c", [H, 1], f32, kind="ExternalInput").ap()
    b2c = nc.dram_tensor("b2c", [OUT, 1], f32, kind="ExternalInput").ap()
    out = nc.dram_tensor("out", [OUT, B_SH], f32, kind="ExternalOutput").ap()

    relu = mybir.ActivationFunctionType.Relu
    ident = mybir.ActivationFunctionType.Identity

    with tile.TileContext(nc) as tc:
        with (
            tc.tile_pool(name="wpool", bufs=1) as wpool,
            tc.tile_pool(name="xpool", bufs=4) as xpool,
            tc.tile_pool(name="hpool", bufs=4) as hpool,
            tc.tile_pool(name="opool", bufs=4) as opool,
            tc.tile_pool(name="ps1", bufs=5, space="PSUM") as ps1pool,
            tc.tile_pool(name="ps2", bufs=2, space="PSUM") as ps2pool,
        ):
            # Replicated params, loaded once (small; keep off the two HWDGE
            # rings that stream x).
            w_sb = wpool.tile([KT, NK, H], f16)
            nc.gpsimd.dma_start(w_sb[:], weff.rearrange("(k p) m -> p k m", p=KT))
            w2_sb = wpool.tile([H, OUT], f32r)
            nc.gpsimd.dma_start(w2_sb[:], w2t[:])
            b1_sb = wpool.tile([H, 1], f32)
            nc.gpsimd.dma_start(b1_sb[:], b1c[:])
            b2_sb = wpool.tile([OUT, 1], f32)
            nc.gpsimd.dma_start(b2_sb[:], b2c[:])

            for g in range(NT // G):
                x_sb = xpool.tile([KT, G, NK, NB], f16)
                # alternate the two HWDGE rings (sync / scalar)
                dma_eng = nc.sync if g % 2 == 0 else nc.scalar
                dma_eng.dma_start(x_sb[:], xtp[:, g * G:(g + 1) * G, :, :])

                for s in range(G):
                    t = g * G + s
                    # h.T[128, NB] = W_eff.T @ x.T, accumulated over K-tiles.
                    ps1 = ps1pool.tile([H, NB], f32)
                    for k in range(NK):
                        nc.tensor.matmul(
                            ps1[:],
                            lhsT=w_sb[:, k, :],
                            rhs=x_sb[:, s, k, :],
                            start=(k == 0),
                            stop=(k == NK - 1),
                        )
                    h_sb = hpool.tile([H, NB], f32r)
                    nc.scalar.activation(h_sb[:], ps1[:], relu, bias=b1_sb[:])

                    # out.T[10, NB] = w2 @ h.T
                    ps2 = ps2pool.tile([OUT, NB], f32)
                    nc.tensor.matmul(ps2[:], lhsT=w2_sb[:], rhs=h_sb[:],
                                     start=True, stop=True)
                    o_sb = opool.tile([OUT, NB], f32)
                    nc.scalar.activation(o_sb[:], ps2[:], ident, bias=b2_sb[:])
                    nc.gpsimd.dma_start(out[:, t * NB:(t + 1) * NB], o_sb[:])

    nc.compile()
    return nc


def _get_nc():
    if "nc" not in _CACHE:
        _CACHE["nc"] = _build_nc()
    return _CACHE["nc"]


def _round_fp32r(a: np.ndarray) -> np.ndarray:
    """Round fp32 to the fp32r format: 11 mantissa bits, RNE, low 12 bits zero."""
    u = np.ascontiguousarray(a, np.float32).view(np.uint32)
    lsb = (u >> np.uint32(12)) & np.uint32(1)
    u = (u + np.uint32(0x7FF) + lsb) & np.uint32(0xFFFFF000)
    return u.view(np.float32)


def _fold_weights(conv_w: np.ndarray, w1: np.ndarray) -> np.ndarray:
    """W_eff[784, 128]: h_pre = x @ W_eff  ==  conv(x) @ w1.T  (float64 accum)."""
    w1k = w1.reshape(H, 26, 26).transpose(1, 2, 0).astype(np.float64)  # [i,j,k]
    cw = conv_w.astype(np.float64)
    W = np.zeros((28, 28, H), np.float64)
    for di in range(3):
        for dj in range(3):
            W[di:di + 26, dj:dj + 26, :] += cw[di, dj] * w1k
    return W.reshape(D, H).astype(np.float32)


def make_in_maps(x, conv_w, w1, b1, w2, b2):
    x = np.asarray(x, np.float32)
    weff = _fold_weights(
        np.asarray(conv_w, np.float32), np.asarray(w1, np.float32)).astype(np.float16)
    w2t = _round_fp32r(np.asarray(w2, np.float32).T)                  # [128, 10]
    b1c = np.ascontiguousarray(np.asarray(b1, np.float32).reshape(H, 1))
    b2c = np.ascontiguousarray(np.asarray(b2, np.float32).reshape(OUT, 1))
    in_maps = []
    for i in range(N_CORES):
        xs = x[i * B_SH:(i + 1) * B_SH]                    # [8192, 784]
        # [t*NB+c, k*KT+p] -> [p, t, k, c]
        xtp = xs.reshape(NT, NB, NK, KT).transpose(3, 0, 2, 1)
        in_maps.append({"xtp": np.ascontiguousarray(xtp, np.float16),
                        "weff": weff, "w2t": w2t, "b1c": b1c, "b2c": b2c})
    return in_maps


def kernel(x, conv_w, w1, b1, w2, b2):
    nc = _get_nc()
    in_maps = make_in_maps(x, conv_w, w1, b1, w2, b2)
    res = run_bass_kernel_spmd(nc, in_maps, list(range(N_CORES)))
    out = np.concatenate([res.results[i]["out"] for i in range(N_CORES)], axis=1)
    return np.ascontiguousarray(out.T)  # [65536, 10] float32


# revision 5
# speedup vs baseline: 1.7804x; 1.0633x over previous
"""Trainium2 Bass kernel for DigitConvolutionalModel.

Math: the 3x3 valid conv on the 28x28 image is a linear map, so it folds into
the first Linear layer:
    out = relu(x @ W_eff + b1) @ w2.T + b2
where W_eff[784, 128] = C @ w1.T and C[784, 676] is the conv-as-matrix built
from conv_w.  W_eff is built on the host (O(1) w.r.t. batch); the device does
the two batch matmuls.

Distribution: pure data parallel — batch dim of x sharded across 8 NeuronCores,
weights replicated.  Each core receives its x shard permuted to a
partition-major layout [112, 16, 7, 512] (partition p, batch-tile t, k-tile k,
column c; feature f = k*112 + p) so every DMA lands on SBUF partitions with
long contiguous DRAM runs.  Each core computes out.T [10, 8192] and the host
reassembles [65536, 10].

dtypes: x and W_eff ship as fp16 (10 mantissa bits, comparable to the PE's
fp32r path at 11 bits) — halves the HBM traffic, which is the roofline here,
and fp16 matmuls run at the full 1 cycle/row PE rate.  The hidden activation
is produced as float32r (fp32, 11-bit mantissa) by the ScalarE relu so the
second matmul also runs at full rate; accumulation is always fp32 in PSUM.
"""

import numpy as np

import concourse.bass as bass  # noqa: F401  (bass registers mybir lowerings)
import concourse.mybir as mybir
import concourse.tile as tile
from concourse import bacc
from concourse.bass_utils import run_bass_kernel_spmd

N_CORES = 8
B = 65536
B_SH = B // N_CORES  # 8192 rows per core
D = 784              # 28*28 input features
H = 128              # hidden
OUT = 10
KT = 112             # contraction tile (SBUF partitions used)
NK = D // KT         # 7 K-tiles
NB = 512             # batch columns per tile (= one fp32 PSUM bank)
NT = B_SH // NB      # 16 batch tiles
G = 2                # batch tiles per x DMA (28KB contiguous per partition)

_CACHE = {}


def _build_nc():
    f32 = mybir.dt.float32
    f32r = mybir.dt.float32r
    f16 = mybir.dt.float16
    nc = bacc.Bacc("TRN2", target_bir_lowering=False, debug=False,
                   num_devices=N_CORES)
    # x, partition-major: [p, t, k, c] with feature f = k*112 + p
    xtp = nc.dram_tensor("xtp", [KT, NT, NK, NB], f16,
                         kind="ExternalInput").ap()
    weff = nc.dram_tensor("weff", [D, H], f16, kind="ExternalInput").ap()
    w2t = nc.dram_tensor("w2t", [H, OUT], f32r, kind="ExternalInput").ap()
    b1c = nc.dram_tensor("b1c", [H, 1], f32, kind="ExternalInput").ap()
    b2c = nc.dram_tensor("b2c", [OUT, 1], f32, kind="ExternalInput").ap()
    out = nc.dram_tensor("out", [OUT, B_SH], f32, kind="ExternalOutput").ap()

    relu = mybir.ActivationFunctionType.Relu

    with tile.TileContext(nc) as tc:
        with (
            tc.tile_pool(name="wpool", bufs=1) as wpool,
            tc.tile_pool(name="xpool", bufs=6) as xpool,
            tc.tile_pool(name="hpool", bufs=4) as hpool,
            tc.tile_pool(name="opool", bufs=4) as opool,
            tc.tile_pool(name="ps1", bufs=5, space="PSUM") as ps1pool,
            tc.tile_pool(name="ps2", bufs=2, space="PSUM") as ps2pool,
        ):
            # Replicated params, loaded once, first in the sync ring so the
            # first matmul isn't gated on a slow SWDGE startup.
            w_sb = wpool.tile([KT, NK, H], f16)
            nc.sync.dma_start(w_sb[:], weff.rearrange("(k p) m -> p k m", p=KT))
            w2_sb = wpool.tile([H, OUT], f32r)
            nc.sync.dma_start(w2_sb[:], w2t[:])
            b1_sb = wpool.tile([H, 1], f32)
            nc.sync.dma_start(b1_sb[:], b1c[:])
            b2_sb = wpool.tile([OUT, 1], f32)
            nc.sync.dma_start(b2_sb[:], b2c[:])

            for g in range(NT // G):
                x_sb = xpool.tile([KT, G, NK, NB], f16)
                # x streams on the sync HWDGE ring only — the scalar ring's
                # trigger instructions would queue behind ACT's relu work.
                nc.sync.dma_start(x_sb[:], xtp[:, g * G:(g + 1) * G, :, :])

                for s in range(G):
                    t = g * G + s
                    # h.T[128, NB] = W_eff.T @ x.T, accumulated over K-tiles.
                    ps1 = ps1pool.tile([H, NB], f32)
                    for k in range(NK):
                        nc.tensor.matmul(
                            ps1[:],
                            lhsT=w_sb[:, k, :],
                            rhs=x_sb[:, s, k, :],
                            start=(k == 0),
                            stop=(k == NK - 1),
                        )
                    h_sb = hpool.tile([H, NB], f32r)
                    nc.scalar.activation(h_sb[:], ps1[:], relu, bias=b1_sb[:])

                    # out.T[10, NB] = w2 @ h.T
                    ps2 = ps2pool.tile([OUT, NB], f32)
                    nc.tensor.matmul(ps2[:], lhsT=w2_sb[:], rhs=h_sb[:],
                                     start=True, stop=True)
                    o_sb = opool.tile([OUT, NB], f32)
                    # bias add on the (otherwise idle) DVE; store via the
                    # scalar HWDGE ring (ACT's trigger cost is tiny).
                    nc.vector.tensor_scalar_add(o_sb[:], ps2[:], b2_sb[:])
                    nc.scalar.dma_start(out[:, t * NB:(t + 1) * NB], o_sb[:])

    nc.compile()
    return nc


def _get_nc():
    if "nc" not in _CACHE:
        _CACHE["nc"] = _build_nc()
    return _CACHE["nc"]


def _round_fp32r(a: np.ndarray) -> np.ndarray:
    """Round fp32 to the fp32r format: 11 mantissa bits, RNE, low 12 bits zero."""
    u = np.ascontiguousarray(a, np.float32).view(np.uint32)
    lsb = (u >> np.uint32(12)) & np.uint32(1)
    u = (u + np.uint32(0x7FF) + lsb) & np.uint32(0xFFFFF000)
    return u.view(np.float32)


def _fold_weights(conv_w: np.ndarray, w1: np.ndarray) -> np.ndarray:
    """W_eff[784, 128]: h_pre = x @ W_eff  ==  conv(x) @ w1.T  (float64 accum)."""
    w1k = w1.reshape(H, 26, 26).transpose(1, 2, 0).astype(np.float64)  # [i,j,k]
    cw = conv_w.astype(np.float64)
    W = np.zeros((28, 28, H), np.float64)
    for di in range(3):
        for dj in range(3):
            W[di:di + 26, dj:dj + 26, :] += cw[di, dj] * w1k
    return W.reshape(D, H).astype(np.float32)


def make_in_maps(x, conv_w, w1, b1, w2, b2):
    x = np.asarray(x, np.float32)
    weff = _fold_weights(
        np.asarray(conv_w, np.float32), np.asarray(w1, np.float32)).astype(np.float16)
    w2t = _round_fp32r(np.asarray(w2, np.float32).T)                  # [128, 10]
    b1c = np.ascontiguousarray(np.asarray(b1, np.float32).reshape(H, 1))
    b2c = np.ascontiguousarray(np.asarray(b2, np.float32).reshape(OUT, 1))
    in_maps = []
    for i in range(N_CORES):
        xs = x[i * B_SH:(i + 1) * B_SH]                    # [8192, 784]
        # [t*NB+c, k*KT+p] -> [p, t, k, c]
        xtp = xs.reshape(NT, NB, NK, KT).transpose(3, 0, 2, 1)
        in_maps.append({"xtp": np.ascontiguousarray(xtp, np.float16),
                        "weff": weff, "w2t": w2t, "b1c": b1c, "b2c": b2c})
    return in_maps


def kernel(x, conv_w, w1, b1, w2, b2):
    nc = _get_nc()
    in_maps = make_in_maps(x, conv_w, w1, b1, w2, b2)
    res = run_bass_kernel_spmd(nc, in_maps, list(range(N_CORES)))
    out = np.concatenate([res.results[i]["out"] for i in range(N_CORES)], axis=1)
    return np.ascontiguousarray(out.T)  # [65536, 10] float32


# revision 6
# speedup vs baseline: 1.9461x; 1.0931x over previous
"""Trainium2 Bass kernel for DigitConvolutionalModel.

Math: the 3x3 valid conv on the 28x28 image is a linear map, so it folds into
the first Linear layer:
    out = relu(x @ W_eff + b1) @ w2.T + b2
where W_eff[784, 128] = C @ w1.T and C[784, 676] is the conv-as-matrix built
from conv_w.  W_eff is built on the host (O(1) w.r.t. batch); the device does
the two batch matmuls.

Distribution: pure data parallel — batch dim of x sharded across 8 NeuronCores,
weights replicated.  Each core receives its x shard permuted to a
partition-major layout [112, 16, 7, 512] (partition p, batch-tile t, k-tile k,
column c; feature f = k*112 + p) so every DMA lands on SBUF partitions with
long contiguous DRAM runs.  Each core computes out.T [10, 8192] and the host
reassembles [65536, 10].

dtypes: x and W_eff ship as fp16 (10 mantissa bits, comparable to the PE's
fp32r path at 11 bits) — halves the HBM traffic, which is the roofline here,
and fp16 matmuls run at the full 1 cycle/row PE rate.  Accumulation is fp32 in
PSUM; the hidden activation is computed on the DVE (fused bias+relu) and
emitted as fp16 for the second matmul.  ScalarE runs no compute at all so the
scalar HWDGE ring is free to stream x in parallel with the sync ring (one
HWDGE ring tops out around 290 GB/s; two get close to the HBM limit).
"""

import numpy as np

import concourse.bass as bass  # noqa: F401  (bass registers mybir lowerings)
import concourse.mybir as mybir
import concourse.tile as tile
from concourse import bacc
from concourse.bass_utils import run_bass_kernel_spmd

N_CORES = 8
B = 65536
B_SH = B // N_CORES  # 8192 rows per core
D = 784              # 28*28 input features
H = 128              # hidden
OUT = 10
KT = 112             # contraction tile (SBUF partitions used)
NK = D // KT         # 7 K-tiles
NB = 512             # batch columns per tile (= one fp32 PSUM bank)
NT = B_SH // NB      # 16 batch tiles
G = 1                # batch tiles per x DMA

_CACHE = {}


def _build_nc():
    f32 = mybir.dt.float32
    f32r = mybir.dt.float32r
    f16 = mybir.dt.float16
    nc = bacc.Bacc("TRN2", target_bir_lowering=False, debug=False,
                   num_devices=N_CORES)
    # x, partition-major: [p, t, k, c] with feature f = k*112 + p
    xtp = nc.dram_tensor("xtp", [KT, NT, NK, NB], f16,
                         kind="ExternalInput").ap()
    weff = nc.dram_tensor("weff", [D, H], f16, kind="ExternalInput").ap()
    w2t = nc.dram_tensor("w2t", [H, OUT], f16, kind="ExternalInput").ap()
    b1c = nc.dram_tensor("b1c", [H, 1], f32, kind="ExternalInput").ap()
    b2c = nc.dram_tensor("b2c", [OUT, 1], f32, kind="ExternalInput").ap()
    out = nc.dram_tensor("out", [OUT, B_SH], f32, kind="ExternalOutput").ap()

    with tile.TileContext(nc) as tc:
        with (
            tc.tile_pool(name="wpool", bufs=1) as wpool,
            tc.tile_pool(name="xpool", bufs=8) as xpool,
            tc.tile_pool(name="hpool", bufs=4) as hpool,
            tc.tile_pool(name="opool", bufs=4) as opool,
            tc.tile_pool(name="ps1", bufs=5, space="PSUM") as ps1pool,
            tc.tile_pool(name="ps2", bufs=2, space="PSUM") as ps2pool,
        ):
            # Replicated params, loaded once, split across the two HWDGE
            # rings ahead of the x stream.
            w_sb = wpool.tile([KT, NK, H], f16)
            nc.sync.dma_start(w_sb[:], weff.rearrange("(k p) m -> p k m", p=KT))
            w2_sb = wpool.tile([H, OUT], f16)
            nc.scalar.dma_start(w2_sb[:], w2t[:])
            b1_sb = wpool.tile([H, 1], f32)
            nc.scalar.dma_start(b1_sb[:], b1c[:])
            b2_sb = wpool.tile([OUT, 1], f32)
            nc.scalar.dma_start(b2_sb[:], b2c[:])

            for g in range(NT // G):
                x_sb = xpool.tile([KT, G, NK, NB], f16)
                # x streams on BOTH HWDGE rings (ScalarE does no compute, so
                # its ring triggers issue without delay).
                dma_eng = nc.sync if g % 2 == 0 else nc.scalar
                dma_eng.dma_start(x_sb[:], xtp[:, g * G:(g + 1) * G, :, :])

                for s in range(G):
                    t = g * G + s
                    # h.T[128, NB] = W_eff.T @ x.T, accumulated over K-tiles.
                    ps1 = ps1pool.tile([H, NB], f32)
                    for k in range(NK):
                        nc.tensor.matmul(
                            ps1[:],
                            lhsT=w_sb[:, k, :],
                            rhs=x_sb[:, s, k, :],
                            start=(k == 0),
                            stop=(k == NK - 1),
                        )
                    # h = relu(ps1 + b1), fused on DVE, emitted as fp16
                    h_sb = hpool.tile([H, NB], f16)
                    nc.vector.tensor_scalar(
                        h_sb[:], ps1[:], b1_sb[:], 0.0,
                        mybir.AluOpType.add, mybir.AluOpType.max)

                    # out.T[10, NB] = w2 @ h.T
                    ps2 = ps2pool.tile([OUT, NB], f32)
                    nc.tensor.matmul(ps2[:], lhsT=w2_sb[:], rhs=h_sb[:],
                                     start=True, stop=True)
                    o_sb = opool.tile([OUT, NB], f32)
                    nc.vector.tensor_scalar_add(o_sb[:], ps2[:], b2_sb[:])
                    # out stores on SWDGE, keeping both HWDGE rings pure-x
                    nc.gpsimd.dma_start(out[:, t * NB:(t + 1) * NB], o_sb[:])

    nc.compile()
    return nc


def _get_nc():
    if "nc" not in _CACHE:
        _CACHE["nc"] = _build_nc()
    return _CACHE["nc"]


def _round_fp32r(a: np.ndarray) -> np.ndarray:
    """Round fp32 to the fp32r format: 11 mantissa bits, RNE, low 12 bits zero."""
    u = np.ascontiguousarray(a, np.float32).view(np.uint32)
    lsb = (u >> np.uint32(12)) & np.uint32(1)
    u = (u + np.uint32(0x7FF) + lsb) & np.uint32(0xFFFFF000)
    return u.view(np.float32)


def _fold_weights(conv_w: np.ndarray, w1: np.ndarray) -> np.ndarray:
    """W_eff[784, 128]: h_pre = x @ W_eff  ==  conv(x) @ w1.T  (float64 accum)."""
    w1k = w1.reshape(H, 26, 26).transpose(1, 2, 0).astype(np.float64)  # [i,j,k]
    cw = conv_w.astype(np.float64)
    W = np.zeros((28, 28, H), np.float64)
    for di in range(3):
        for dj in range(3):
            W[di:di + 26, dj:dj + 26, :] += cw[di, dj] * w1k
    return W.reshape(D, H).astype(np.float32)


def make_in_maps(x, conv_w, w1, b1, w2, b2):
    x = np.asarray(x, np.float32)
    weff = _fold_weights(
        np.asarray(conv_w, np.float32), np.asarray(w1, np.float32)).astype(np.float16)
    w2t = np.ascontiguousarray(np.asarray(w2, np.float32).T).astype(np.float16)
    b1c = np.ascontiguousarray(np.asarray(b1, np.float32).reshape(H, 1))
    b2c = np.ascontiguousarray(np.asarray(b2, np.float32).reshape(OUT, 1))
    in_maps = []
    for i in range(N_CORES):
        xs = x[i * B_SH:(i + 1) * B_SH]                    # [8192, 784]
        # [t*NB+c, k*KT+p] -> [p, t, k, c]
        xtp = xs.reshape(NT, NB, NK, KT).transpose(3, 0, 2, 1)
        in_maps.append({"xtp": np.ascontiguousarray(xtp, np.float16),
                        "weff": weff, "w2t": w2t, "b1c": b1c, "b2c": b2c})
    return in_maps


def kernel(x, conv_w, w1, b1, w2, b2):
    nc = _get_nc()
    in_maps = make_in_maps(x, conv_w, w1, b1, w2, b2)
    res = run_bass_kernel_spmd(nc, in_maps, list(range(N_CORES)))
    out = np.concatenate([res.results[i]["out"] for i in range(N_CORES)], axis=1)
    return np.ascontiguousarray(out.T)  # [65536, 10] float32
